# revision 25
# baseline (speedup 1.0000x reference)
"""Trainium2 Bass kernel for nn_CrossEntropyMoreToMore.

Math: out[i, n] = sum_c softplus(pre_cls[n, c]) - pre_cls[n, gt_kind_ind[i]]
with M = N = 8192, C = 80.

Key structure: the output has only C=80 distinct rows. Define
    D[c, n] = base[n] - pre_cls[n, c],  base[n] = sum_c softplus(pre_cls[n, c])
then out[i, :] = D[g[i], :].

Per-core plan (core k owns output rows [k*1024, (k+1)*1024)) — pure DMA
broadcast, no matmul in the main loop:
  1. Build D [80 part, 8192 free] f32 in SBUF, pipelined in column quarters:
     load pre_cls chunk -> softplus (Abs/Exp/Ln compose) -> reduce ->
     subtract -> PE-transpose -> copy.
  2. Build the occurrence table T[c, k] = output row index of the k-th
     occurrence of class c in this core's shard (OOB sentinel when the class
     has fewer than k+1 occurrences):
       - one-hot rows per m-tile (iota + is_equal)
       - rank(m) = # earlier rows of the same class, via two matmuls per
         m-tile (strict-upper-triangular prefix + running column sums)
       - flat index g*KMAX + rank, scattered into a DRAM table by indirect
         DMA, then loaded back as [80, KMAX].
  3. For each k < KMAX: one indirect scatter-DMA broadcasts SBUF row D[c]
     to output row T[c, k] for every class simultaneously (rows with OOB
     index are skipped). Split into column halves so scattering of half 0
     overlaps the build of half 1.

HBM traffic per core = 32 MB output writes + 2.6 MB input reads; the write
stream goes straight SBUF -> DRAM at the DMA roofline (~90 us/core).
"""

import os

import numpy as np

M, N, C = 8192, 8192, 80
N_CORES = 8
M_SHARD = M // N_CORES  # 1024 output rows per core
P = 128  # partitions
NT = N // P  # 64 column tiles of pre_cls
MT = M_SHARD // P  # 8 m-tiles per core
NQ = 4  # column quarters for the pipelined table build
QT = NT // NQ  # 16 transpose tiles per quarter
QW = N // NQ  # 2048 columns per quarter

KMAX = int(os.environ.get("KMAX", "48"))  # max occurrences of one class
NSPLIT = int(os.environ.get("NSPLIT", "2"))  # column splits of the scatter
OOB = 1 << 28
_SKIP_TSCAT = os.environ.get("SKIP_TSCAT") == "1"  # debug bisection
_SKIP_MAIN = os.environ.get("SKIP_MAIN") == "1"  # debug bisection

_compiled_nc = None


def _build_kernel():
    import concourse.bacc as bacc
    import concourse.mybir as mybir
    import concourse.tile as tile
    from concourse import bass
    from concourse.masks import make_identity, make_upper_triangular

    nc = bacc.Bacc(
        "TRN2",
        target_bir_lowering=False,
        debug=False,
        num_devices=N_CORES,
    )
    fp32 = mybir.dt.float32
    i32 = mybir.dt.int32
    AF = mybir.ActivationFunctionType
    ALU = mybir.AluOpType

    g_dram = nc.dram_tensor("g", [M_SHARD], fp32, kind="ExternalInput")
    pre_dram = nc.dram_tensor("pre", [N, C], fp32, kind="ExternalInput")
    out_dram = nc.dram_tensor("out", [M_SHARD, N], fp32, kind="ExternalOutput")
    # ExternalOutput (not Internal): internal DRAM scratch may be paged by
    # the compiler, which breaks indirect row addressing into it.
    t_dram = nc.dram_tensor("t_scratch", [C * KMAX, 1], i32, kind="ExternalOutput")

    pre_tiled = pre_dram.ap().rearrange("(t p) c -> p t c", p=P)

    with tile.TileContext(nc) as tc:
        with (
            tc.tile_pool(name="setup", bufs=1) as setup,
            tc.tile_pool(name="pipe", bufs=2) as pipe,
            tc.tile_pool(name="psum", bufs=2, space="PSUM") as psum,
        ):
            ident = setup.tile([P, P], fp32)
            make_identity(nc, ident[:])
            u_strict = setup.tile([P, P], fp32)
            # u[k, m] = 1 iff k < m  (strictly-lower in (row, col) = (k, m))
            make_upper_triangular(nc, u_strict[:], val=1.0, diag=False)
            ones_sq = setup.tile([P, P], fp32)
            nc.vector.memset(ones_sq[:], 1.0)

            # ---- occurrence table T[c, k] ----
            g_col = setup.tile([P, MT], fp32)
            nc.sync.dma_start(g_col[:], g_dram.ap().rearrange("(t p) -> p t", p=P))
            iota_row = setup.tile([P, C], fp32)
            nc.gpsimd.iota(
                iota_row[:],
                pattern=[[1, C]],
                channel_multiplier=0,
                allow_small_or_imprecise_dtypes=True,
            )
            iota_m = setup.tile([P, MT], i32)
            nc.gpsimd.iota(iota_m[:], pattern=[[P, MT]], channel_multiplier=1)

            # prefill T with OOB sentinel
            sent = setup.tile([C, KMAX], i32)
            nc.vector.memset(sent[:], OOB)
            nc.sync.dma_start(
                t_dram.ap().rearrange("(c k) o -> c (k o)", c=C), sent[:]
            )

            _SKIP_RANK = os.environ.get("SKIP_RANK") == "1"
            def strip_intra_deps(insts):
                """Remove scatter<->scatter WAW edges: each indirect scatter
                in a group writes a disjoint set of rows (distinct flat
                indices / each output row written exactly once), so Tile's
                conservative same-tensor serialization is unnecessary and
                would serialize the whole DMA stream."""
                names = {i.ins.name for i in insts}
                for ins_ in insts:
                    for d in list(ins_.ins.sync_dependency_names()):
                        if d in names:
                            ins_.ins.try_remove_dependency(d)

            tscat_insts = []
            cum = setup.tile([P, C], fp32)
            nc.vector.memset(cum[:], 0.0)
            for i in range(MT if not _SKIP_RANK else 0):
                rowhot = pipe.tile([P, C], fp32, tag="rowhot")
                nc.vector.tensor_scalar(
                    out=rowhot[:],
                    in0=iota_row[:],
                    scalar1=g_col[:, i : i + 1],
                    scalar2=None,
                    op0=ALU.is_equal,
                )
                ps = psum.tile([P, C], fp32, tag="rank")
                nc.tensor.matmul(
                    ps[:], lhsT=u_strict[:], rhs=rowhot[:], start=True, stop=False
                )
                nc.tensor.matmul(
                    ps[:], lhsT=ones_sq[:], rhs=cum[:], start=False, stop=True
                )
                nc.vector.tensor_add(cum[:], cum[:], rowhot[:])
                # rank[m] = sum_c ps[m, c] * rowhot[m, c]
                scr = pipe.tile([P, C], fp32, tag="scr")
                rank = pipe.tile([P, 1], fp32, tag="rank_col")
                nc.vector.tensor_tensor(
                    out=scr[:], in0=ps[:], in1=rowhot[:], op=ALU.mult
                )
                nc.vector.reduce_sum(rank[:], scr[:], axis=mybir.AxisListType.X)
                # flat = g*KMAX + rank, OOB when rank >= KMAX
                ovf = pipe.tile([P, 1], fp32, tag="ovf")
                nc.vector.tensor_scalar(
                    out=ovf[:],
                    in0=rank[:],
                    scalar1=float(KMAX) - 0.5,
                    scalar2=None,
                    op0=ALU.is_gt,
                )
                flat_f = pipe.tile([P, 1], fp32, tag="flat_f")
                nc.vector.tensor_scalar(
                    out=flat_f[:],
                    in0=g_col[:, i : i + 1],
                    scalar1=float(KMAX),
                    scalar2=rank[:, :1],
                    op0=ALU.mult,
                    op1=ALU.add,
                )
                nc.vector.tensor_scalar(
                    out=flat_f[:],
                    in0=ovf[:],
                    scalar1=float(OOB),
                    scalar2=flat_f[:, :1],
                    op0=ALU.mult,
                    op1=ALU.add,
                )
                flat_i = pipe.tile([P, 1], i32, tag="flat_i")
                nc.vector.tensor_copy(flat_i[:], flat_f[:])
                if not _SKIP_TSCAT:
                    tscat_insts.append(
                        nc.gpsimd.indirect_dma_start(
                            out=t_dram.ap(),
                            out_offset=bass.IndirectOffsetOnAxis(
                                ap=flat_i[:, :1], axis=0
                            ),
                            in_=iota_m[:, i : i + 1],
                            in_offset=None,
                            bounds_check=C * KMAX - 1,
                            oob_is_err=False,
                        )
                    )
            strip_intra_deps(tscat_insts)
            t_sb = setup.tile([C, KMAX], i32)
            nc.sync.dma_start(
                t_sb[:], t_dram.ap().rearrange("(c k) o -> c (k o)", c=C)
            )

            # ---- D table [80, 8192] f32, built in column quarters ----
            d_table = setup.tile([C, N], fp32)
            for Q in range(NQ):
                pre_q = pipe.tile([P, QT, C], fp32, tag="pre")
                nc.sync.dma_start(pre_q[:], pre_tiled[:, Q * QT : (Q + 1) * QT, :])
                # softplus(x) = relu(x) + ln(1 + exp(-|x|))
                t0 = pipe.tile([P, QT, C], fp32, tag="t0")
                nc.scalar.activation(t0[:], pre_q[:], AF.Abs)
                nc.scalar.activation(t0[:], t0[:], AF.Exp, scale=-1.0)
                nc.scalar.activation(t0[:], t0[:], AF.Ln, bias=1.0)
                rx = pipe.tile([P, QT, C], fp32, tag="rx")
                nc.vector.tensor_scalar_max(rx[:], pre_q[:], 0.0)
                nc.vector.tensor_add(rx[:], t0[:], rx[:])  # rx = softplus(pre)
                baseq = pipe.tile([P, QT, 1], fp32, tag="base")
                nc.vector.reduce_sum(baseq[:], rx[:], axis=mybir.AxisListType.X)
                # dtt[p, t, c] = base[p, t] - pre[p, t, c]  (onto t0)
                nc.vector.tensor_tensor(
                    out=t0[:],
                    in0=baseq[:].to_broadcast([P, QT, C]),
                    in1=pre_q[:],
                    op=ALU.subtract,
                )
                for t in range(QT):
                    ps = psum.tile([C, P], fp32, tag="tr")
                    nc.tensor.transpose(ps[:], t0[:, t, :], ident[:])
                    tp = (Q * QT + t) * P
                    nc.scalar.copy(d_table[:, tp : tp + P], ps[:])

            # ---- broadcast scatters: row D[c] -> out row T[c, k] ----
            mscat_insts = []
            CW = N // NSPLIT
            for half in range(NSPLIT if not _SKIP_MAIN else 0):
                cols = slice(half * CW, (half + 1) * CW)
                for k in range(KMAX):
                    mscat_insts.append(
                        nc.gpsimd.indirect_dma_start(
                            out=out_dram.ap()[:, cols],
                            out_offset=bass.IndirectOffsetOnAxis(
                                ap=t_sb[:, k : k + 1], axis=0
                            ),
                            in_=d_table[:, cols],
                            in_offset=None,
                            bounds_check=M_SHARD - 1,
                            oob_is_err=False,
                        )
                    )
            strip_intra_deps(mscat_insts)

    nc.compile()
    return nc


def _get_nc():
    global _compiled_nc
    if _compiled_nc is None:
        _compiled_nc = _build_kernel()
    return _compiled_nc


def _in_maps(gt_kind_ind, pre_cls):
    g = np.ascontiguousarray(np.asarray(gt_kind_ind).astype(np.float32))
    pre = np.ascontiguousarray(np.asarray(pre_cls, dtype=np.float32))
    assert g.shape == (M,) and pre.shape == (N, C)
    return [
        {"g": g[k * M_SHARD : (k + 1) * M_SHARD], "pre": pre}
        for k in range(N_CORES)
    ]


def kernel(gt_kind_ind, pre_cls, _trace=False):
    from concourse.bass_utils import run_bass_kernel_spmd

    nc = _get_nc()
    res = run_bass_kernel_spmd(
        nc, _in_maps(gt_kind_ind, pre_cls), list(range(N_CORES)), trace=_trace
    )
    out = np.concatenate(
        [res.results[k]["out"] for k in range(N_CORES)], axis=0
    )
    if _trace:
        return out, res
    return out


# revision 30
# speedup vs baseline: 1.0547x; 1.0547x over previous
"""Trainium2 Bass kernel for nn_CrossEntropyMoreToMore.

Math: out[i, n] = sum_c softplus(pre_cls[n, c]) - pre_cls[n, gt_kind_ind[i]]
with M = N = 8192, C = 80.

Key structure: the output has only C=80 distinct rows. Define
    D[c, n] = base[n] - pre_cls[n, c],  base[n] = sum_c softplus(pre_cls[n, c])
then out[i, :] = D[g[i], :].

Per-core plan (core k owns output rows [k*1024, (k+1)*1024)) — pure DMA
broadcast, no matmul in the main loop:
  1. Build D [80 part, 8192 free] f32 in SBUF, pipelined in column quarters:
     load pre_cls chunk -> softplus (Abs/Exp/Ln compose) -> reduce ->
     subtract -> PE-transpose -> copy.
  2. Build the occurrence table T[c, k] = output row index of the k-th
     occurrence of class c in this core's shard (OOB sentinel when the class
     has fewer than k+1 occurrences):
       - one-hot rows per m-tile (iota + is_equal)
       - rank(m) = # earlier rows of the same class, via two matmuls per
         m-tile (strict-upper-triangular prefix + running column sums)
       - flat index g*KMAX + rank, scattered into a DRAM table by indirect
         DMA, then loaded back as [80, KMAX].
  3. For each k < KMAX: one indirect scatter-DMA broadcasts SBUF row D[c]
     to output row T[c, k] for every class simultaneously (rows with OOB
     index are skipped). Split into column halves so scattering of half 0
     overlaps the build of half 1.

HBM traffic per core = 32 MB output writes + 2.6 MB input reads; the write
stream goes straight SBUF -> DRAM at the DMA roofline (~90 us/core).
"""

import os

import numpy as np

M, N, C = 8192, 8192, 80
N_CORES = 8
M_SHARD = M // N_CORES  # 1024 output rows per core
P = 128  # partitions
NT = N // P  # 64 column tiles of pre_cls
MT = M_SHARD // P  # 8 m-tiles per core
NQ = 4  # column quarters for the pipelined table build
QT = NT // NQ  # 16 transpose tiles per quarter
QW = N // NQ  # 2048 columns per quarter

KMAX = int(os.environ.get("KMAX", "36"))  # max occurrences of one class
NSPLIT = int(os.environ.get("NSPLIT", "2"))  # column splits of the scatter
OOB = 1 << 28
_SKIP_TSCAT = os.environ.get("SKIP_TSCAT") == "1"  # debug bisection
_SKIP_MAIN = os.environ.get("SKIP_MAIN") == "1"  # debug bisection

_compiled_nc = None


def _build_kernel():
    import concourse.bacc as bacc
    import concourse.mybir as mybir
    import concourse.tile as tile
    from concourse import bass
    from concourse.masks import make_identity, make_upper_triangular

    nc = bacc.Bacc(
        "TRN2",
        target_bir_lowering=False,
        debug=False,
        num_devices=N_CORES,
    )
    fp32 = mybir.dt.float32
    i32 = mybir.dt.int32
    AF = mybir.ActivationFunctionType
    ALU = mybir.AluOpType

    g_dram = nc.dram_tensor("g", [M_SHARD], fp32, kind="ExternalInput")
    pre_dram = nc.dram_tensor("pre", [N, C], fp32, kind="ExternalInput")
    out_dram = nc.dram_tensor("out", [M_SHARD, N], fp32, kind="ExternalOutput")
    # ExternalOutput (not Internal): internal DRAM scratch may be paged by
    # the compiler, which breaks indirect row addressing into it.
    t_dram = nc.dram_tensor("t_scratch", [C * KMAX, 1], i32, kind="ExternalOutput")

    pre_tiled = pre_dram.ap().rearrange("(t p) c -> p t c", p=P)

    with tile.TileContext(nc) as tc:
        with (
            tc.tile_pool(name="setup", bufs=1) as setup,
            tc.tile_pool(name="pipe", bufs=2) as pipe,
            tc.tile_pool(name="psum", bufs=2, space="PSUM") as psum,
        ):
            ident = setup.tile([P, P], fp32)
            make_identity(nc, ident[:])
            u_strict = setup.tile([P, P], fp32)
            # u[k, m] = 1 iff k < m  (strictly-lower in (row, col) = (k, m))
            make_upper_triangular(nc, u_strict[:], val=1.0, diag=False)
            ones_sq = setup.tile([P, P], fp32)
            nc.vector.memset(ones_sq[:], 1.0)

            # ---- occurrence table T[c, k] ----
            g_col = setup.tile([P, MT], fp32)
            nc.sync.dma_start(g_col[:], g_dram.ap().rearrange("(t p) -> p t", p=P))
            iota_row = setup.tile([P, C], fp32)
            nc.gpsimd.iota(
                iota_row[:],
                pattern=[[1, C]],
                channel_multiplier=0,
                allow_small_or_imprecise_dtypes=True,
            )
            iota_m = setup.tile([P, MT], i32)
            nc.gpsimd.iota(iota_m[:], pattern=[[P, MT]], channel_multiplier=1)

            # prefill T with OOB sentinel
            sent = setup.tile([C, KMAX], i32)
            nc.vector.memset(sent[:], OOB)
            nc.sync.dma_start(
                t_dram.ap().rearrange("(c k) o -> c (k o)", c=C), sent[:]
            )

            _SKIP_RANK = os.environ.get("SKIP_RANK") == "1"
            def strip_intra_deps(insts):
                """Remove scatter<->scatter WAW edges: each indirect scatter
                in a group writes a disjoint set of rows (distinct flat
                indices / each output row written exactly once), so Tile's
                conservative same-tensor serialization is unnecessary and
                would serialize the whole DMA stream."""
                names = {i.ins.name for i in insts}
                for ins_ in insts:
                    for d in list(ins_.ins.sync_dependency_names()):
                        if d in names:
                            ins_.ins.try_remove_dependency(d)

            bc_t = nc.gpsimd.to_reg(C * KMAX - 1)
            bc_m = nc.gpsimd.to_reg(M_SHARD - 1)
            tscat_insts = []
            cum = setup.tile([P, C], fp32)
            nc.vector.memset(cum[:], 0.0)
            for i in range(MT if not _SKIP_RANK else 0):
                rowhot = pipe.tile([P, C], fp32, tag="rowhot")
                nc.vector.tensor_scalar(
                    out=rowhot[:],
                    in0=iota_row[:],
                    scalar1=g_col[:, i : i + 1],
                    scalar2=None,
                    op0=ALU.is_equal,
                )
                ps = psum.tile([P, C], fp32, tag="rank")
                nc.tensor.matmul(
                    ps[:], lhsT=u_strict[:], rhs=rowhot[:], start=True, stop=False
                )
                nc.tensor.matmul(
                    ps[:], lhsT=ones_sq[:], rhs=cum[:], start=False, stop=True
                )
                nc.vector.tensor_add(cum[:], cum[:], rowhot[:])
                # rank[m] = sum_c ps[m, c] * rowhot[m, c]
                scr = pipe.tile([P, C], fp32, tag="scr")
                rank = pipe.tile([P, 1], fp32, tag="rank_col")
                nc.vector.tensor_tensor(
                    out=scr[:], in0=ps[:], in1=rowhot[:], op=ALU.mult
                )
                nc.vector.reduce_sum(rank[:], scr[:], axis=mybir.AxisListType.X)
                # flat = g*KMAX + rank, OOB when rank >= KMAX
                ovf = pipe.tile([P, 1], fp32, tag="ovf")
                nc.vector.tensor_scalar(
                    out=ovf[:],
                    in0=rank[:],
                    scalar1=float(KMAX) - 0.5,
                    scalar2=None,
                    op0=ALU.is_gt,
                )
                flat_f = pipe.tile([P, 1], fp32, tag="flat_f")
                nc.vector.tensor_scalar(
                    out=flat_f[:],
                    in0=g_col[:, i : i + 1],
                    scalar1=float(KMAX),
                    scalar2=rank[:, :1],
                    op0=ALU.mult,
                    op1=ALU.add,
                )
                nc.vector.tensor_scalar(
                    out=flat_f[:],
                    in0=ovf[:],
                    scalar1=float(OOB),
                    scalar2=flat_f[:, :1],
                    op0=ALU.mult,
                    op1=ALU.add,
                )
                flat_i = pipe.tile([P, 1], i32, tag="flat_i")
                nc.vector.tensor_copy(flat_i[:], flat_f[:])
                if not _SKIP_TSCAT:
                    tscat_insts.append(
                        nc.gpsimd.indirect_dma_start(
                            out=t_dram.ap(),
                            out_offset=bass.IndirectOffsetOnAxis(
                                ap=flat_i[:, :1], axis=0
                            ),
                            in_=iota_m[:, i : i + 1],
                            in_offset=None,
                            bounds_check=bc_t,
                            oob_is_err=False,
                        )
                    )
            strip_intra_deps(tscat_insts)
            t_sb = setup.tile([C, KMAX], i32)
            nc.sync.dma_start(
                t_sb[:], t_dram.ap().rearrange("(c k) o -> c (k o)", c=C)
            )

            # ---- D table as two column-half tables on different partition
            # ranges, so the scatter DMA source reads spread over all 16 SDMA
            # engines (a single 80-partition table loads only ~10 of them).
            N2 = N // 2
            tab_lo = setup.tile([C, N2], fp32)      # cols [0, N2) @ parts 0-79
            tab_hi = setup.tile([P, N2], fp32)      # cols [N2, N) @ parts 32-111
            tab_hi_st = setup.tile([C, N2], fp32)   # staging @ parts 0-79
            PSH = 32  # partition shift of the high table (PE psum base must be 0/32/64)
            for Q in range(NQ):
                pre_q = pipe.tile([P, QT, C], fp32, tag="pre")
                nc.sync.dma_start(pre_q[:], pre_tiled[:, Q * QT : (Q + 1) * QT, :])
                # softplus(x) = relu(x) + ln(1 + exp(-|x|))
                # |x| on DVE via sign-bit mask; Exp/Ln on ACT.
                t0 = pipe.tile([P, QT, C], fp32, tag="t0")
                nc.scalar.activation(t0[:], pre_q[:], AF.Abs)
                nc.scalar.activation(t0[:], t0[:], AF.Exp, scale=-1.0)
                nc.scalar.activation(t0[:], t0[:], AF.Ln, bias=1.0)
                rx = pipe.tile([P, QT, C], fp32, tag="rx")
                nc.vector.tensor_scalar_max(rx[:], pre_q[:], 0.0)
                nc.vector.tensor_add(rx[:], t0[:], rx[:])  # rx = softplus(pre)
                baseq = pipe.tile([P, QT, 1], fp32, tag="base")
                nc.vector.reduce_sum(baseq[:], rx[:], axis=mybir.AxisListType.X)
                # dtt[p, t, c] = base[p, t] - pre[p, t, c]  (onto t0)
                nc.vector.tensor_tensor(
                    out=t0[:],
                    in0=baseq[:].to_broadcast([P, QT, C]),
                    in1=pre_q[:],
                    op=ALU.subtract,
                )
                hi = Q >= NQ // 2
                for t in range(QT):
                    ps = psum.tile([P, P], fp32, tag="tr")
                    pview = ps[0:C, :]
                    nc.tensor.transpose(pview, t0[:, t, :], ident[:])
                    gt = Q * QT + t
                    if hi:
                        tp = (gt - NT // 2) * P
                        nc.scalar.copy(tab_hi_st[:, tp : tp + P], pview)
                    else:
                        tp = gt * P
                        nc.scalar.copy(tab_lo[:, tp : tp + P], pview)
                if hi:
                    # partition-shift the finished quarter to parts 32-111
                    qs = (Q - NQ // 2) * QW
                    nc.sync.dma_start(
                        tab_hi[PSH : PSH + C, qs : qs + QW],
                        tab_hi_st[:, qs : qs + QW],
                    )

            # ---- broadcast scatters: row D[c] -> out row T[c, k] ----
            mscat_insts = []
            if not _SKIP_MAIN:
                for half in range(2):
                    src = tab_lo[:, :] if half == 0 else tab_hi[PSH : PSH + C, :]
                    for k in range(KMAX):
                        mscat_insts.append(
                            nc.gpsimd.indirect_dma_start(
                                out=out_dram.ap(),
                                out_offset=bass.IndirectOffsetOnAxis(
                                    ap=t_sb[:, k : k + 1], axis=0
                                ),
                                in_=src,
                                in_offset=None,
                                element_offset=half * N2,
                                bounds_check=bc_m,
                                oob_is_err=False,
                            )
                        )
            strip_intra_deps(mscat_insts)

    nc.compile()
    return nc


def _get_nc():
    global _compiled_nc
    if _compiled_nc is None:
        _compiled_nc = _build_kernel()
    return _compiled_nc


def _in_maps(gt_kind_ind, pre_cls):
    g = np.ascontiguousarray(np.asarray(gt_kind_ind).astype(np.float32))
    pre = np.ascontiguousarray(np.asarray(pre_cls, dtype=np.float32))
    assert g.shape == (M,) and pre.shape == (N, C)
    return [
        {"g": g[k * M_SHARD : (k + 1) * M_SHARD], "pre": pre}
        for k in range(N_CORES)
    ]


def kernel(gt_kind_ind, pre_cls, _trace=False):
    from concourse.bass_utils import run_bass_kernel_spmd

    nc = _get_nc()
    res = run_bass_kernel_spmd(
        nc, _in_maps(gt_kind_ind, pre_cls), list(range(N_CORES)), trace=_trace
    )
    out = np.concatenate(
        [res.results[k]["out"] for k in range(N_CORES)], axis=0
    )
    if _trace:
        return out, res
    return out


# revision 31
# speedup vs baseline: 1.1530x; 1.0932x over previous
"""Trainium2 Bass kernel for nn_CrossEntropyMoreToMore.

Math: out[i, n] = sum_c softplus(pre_cls[n, c]) - pre_cls[n, gt_kind_ind[i]]
with M = N = 8192, C = 80.

Key structure: the output has only C=80 distinct rows. Define
    D[c, n] = base[n] - pre_cls[n, c],  base[n] = sum_c softplus(pre_cls[n, c])
then out[i, :] = D[g[i], :].

Per-core plan (core k owns output rows [k*1024, (k+1)*1024)) — pure DMA
broadcast, no matmul in the main loop:
  1. Build D [80 part, 8192 free] f32 in SBUF, pipelined in column quarters:
     load pre_cls chunk -> softplus (Abs/Exp/Ln compose) -> reduce ->
     subtract -> PE-transpose -> copy.
  2. Build the occurrence table T[c, k] = output row index of the k-th
     occurrence of class c in this core's shard (OOB sentinel when the class
     has fewer than k+1 occurrences):
       - one-hot rows per m-tile (iota + is_equal)
       - rank(m) = # earlier rows of the same class, via two matmuls per
         m-tile (strict-upper-triangular prefix + running column sums)
       - flat index g*KMAX + rank, scattered into a DRAM table by indirect
         DMA, then loaded back as [80, KMAX].
  3. For each k < KMAX: one indirect scatter-DMA broadcasts SBUF row D[c]
     to output row T[c, k] for every class simultaneously (rows with OOB
     index are skipped). Split into column halves so scattering of half 0
     overlaps the build of half 1.

HBM traffic per core = 32 MB output writes + 2.6 MB input reads; the write
stream goes straight SBUF -> DRAM at the DMA roofline (~90 us/core).
"""

import os

import numpy as np

M, N, C = 8192, 8192, 80
N_CORES = 8
M_SHARD = M // N_CORES  # 1024 output rows per core
P = 128  # partitions
NT = N // P  # 64 column tiles of pre_cls
MT = M_SHARD // P  # 8 m-tiles per core
NQ = 4  # column quarters for the pipelined table build
QT = NT // NQ  # 16 transpose tiles per quarter
QW = N // NQ  # 2048 columns per quarter

KMAX = int(os.environ.get("KMAX", "36"))  # max occurrences of one class
NSPLIT = int(os.environ.get("NSPLIT", "2"))  # column splits of the scatter
OOB = 1 << 28
_SKIP_TSCAT = os.environ.get("SKIP_TSCAT") == "1"  # debug bisection
_SKIP_MAIN = os.environ.get("SKIP_MAIN") == "1"  # debug bisection

_compiled_nc = None


def _build_kernel():
    import concourse.bacc as bacc
    import concourse.mybir as mybir
    import concourse.tile as tile
    from concourse import bass
    from concourse.masks import make_identity, make_upper_triangular

    nc = bacc.Bacc(
        "TRN2",
        target_bir_lowering=False,
        debug=False,
        num_devices=N_CORES,
    )
    fp32 = mybir.dt.float32
    i32 = mybir.dt.int32
    AF = mybir.ActivationFunctionType
    ALU = mybir.AluOpType

    g_dram = nc.dram_tensor("g", [M_SHARD], fp32, kind="ExternalInput")
    pre_dram = nc.dram_tensor("pre", [N, C], fp32, kind="ExternalInput")
    out_dram = nc.dram_tensor("out", [M_SHARD, N], fp32, kind="ExternalOutput")
    # ExternalOutput (not Internal): internal DRAM scratch may be paged by
    # the compiler, which breaks indirect row addressing into it.
    t_dram = nc.dram_tensor("t_scratch", [C * KMAX, 1], i32, kind="ExternalOutput")

    pre_tiled = pre_dram.ap().rearrange("(t p) c -> p t c", p=P)

    with tile.TileContext(nc) as tc:
        with (
            tc.tile_pool(name="setup", bufs=1) as setup,
            tc.tile_pool(name="pipe", bufs=2) as pipe,
            tc.tile_pool(name="psum", bufs=2, space="PSUM") as psum,
        ):
            ident = setup.tile([P, P], fp32)
            make_identity(nc, ident[:])
            u_strict = setup.tile([P, P], fp32)
            # u[k, m] = 1 iff k < m  (strictly-lower in (row, col) = (k, m))
            make_upper_triangular(nc, u_strict[:], val=1.0, diag=False)
            ones_sq = setup.tile([P, P], fp32)
            nc.vector.memset(ones_sq[:], 1.0)

            # ---- occurrence table T[c, k] ----
            g_col = setup.tile([P, MT], fp32)
            nc.sync.dma_start(g_col[:], g_dram.ap().rearrange("(t p) -> p t", p=P))
            iota_row = setup.tile([P, C], fp32)
            nc.gpsimd.iota(
                iota_row[:],
                pattern=[[1, C]],
                channel_multiplier=0,
                allow_small_or_imprecise_dtypes=True,
            )
            iota_m = setup.tile([P, MT], i32)
            nc.gpsimd.iota(iota_m[:], pattern=[[P, MT]], channel_multiplier=1)

            # prefill T with OOB sentinel
            sent = setup.tile([C, KMAX], i32)
            nc.vector.memset(sent[:], OOB)
            nc.sync.dma_start(
                t_dram.ap().rearrange("(c k) o -> c (k o)", c=C), sent[:]
            )

            _SKIP_RANK = os.environ.get("SKIP_RANK") == "1"
            def strip_intra_deps(insts):
                """Remove scatter<->scatter WAW edges: each indirect scatter
                in a group writes a disjoint set of rows (distinct flat
                indices / each output row written exactly once), so Tile's
                conservative same-tensor serialization is unnecessary and
                would serialize the whole DMA stream."""
                names = {i.ins.name for i in insts}
                for ins_ in insts:
                    for d in list(ins_.ins.sync_dependency_names()):
                        if d in names:
                            ins_.ins.try_remove_dependency(d)

            bc_t = nc.gpsimd.to_reg(C * KMAX - 1)
            bc_m = nc.gpsimd.to_reg(M_SHARD - 1)
            tscat_insts = []
            cum = setup.tile([P, C], fp32)
            nc.vector.memset(cum[:], 0.0)
            for i in range(MT if not _SKIP_RANK else 0):
                rowhot = pipe.tile([P, C], fp32, tag="rowhot")
                nc.vector.tensor_scalar(
                    out=rowhot[:],
                    in0=iota_row[:],
                    scalar1=g_col[:, i : i + 1],
                    scalar2=None,
                    op0=ALU.is_equal,
                )
                ps = psum.tile([P, C], fp32, tag="rank")
                nc.tensor.matmul(
                    ps[:], lhsT=u_strict[:], rhs=rowhot[:], start=True, stop=False
                )
                nc.tensor.matmul(
                    ps[:], lhsT=ones_sq[:], rhs=cum[:], start=False, stop=True
                )
                nc.vector.tensor_add(cum[:], cum[:], rowhot[:])
                # rank[m] = sum_c ps[m, c] * rowhot[m, c]
                scr = pipe.tile([P, C], fp32, tag="scr")
                rank = pipe.tile([P, 1], fp32, tag="rank_col")
                nc.vector.tensor_tensor(
                    out=scr[:], in0=ps[:], in1=rowhot[:], op=ALU.mult
                )
                nc.vector.reduce_sum(rank[:], scr[:], axis=mybir.AxisListType.X)
                # flat = g*KMAX + rank, OOB when rank >= KMAX
                ovf = pipe.tile([P, 1], fp32, tag="ovf")
                nc.vector.tensor_scalar(
                    out=ovf[:],
                    in0=rank[:],
                    scalar1=float(KMAX) - 0.5,
                    scalar2=None,
                    op0=ALU.is_gt,
                )
                flat_f = pipe.tile([P, 1], fp32, tag="flat_f")
                nc.vector.tensor_scalar(
                    out=flat_f[:],
                    in0=g_col[:, i : i + 1],
                    scalar1=float(KMAX),
                    scalar2=rank[:, :1],
                    op0=ALU.mult,
                    op1=ALU.add,
                )
                nc.vector.tensor_scalar(
                    out=flat_f[:],
                    in0=ovf[:],
                    scalar1=float(OOB),
                    scalar2=flat_f[:, :1],
                    op0=ALU.mult,
                    op1=ALU.add,
                )
                flat_i = pipe.tile([P, 1], i32, tag="flat_i")
                nc.vector.tensor_copy(flat_i[:], flat_f[:])
                if not _SKIP_TSCAT:
                    tscat_insts.append(
                        nc.gpsimd.indirect_dma_start(
                            out=t_dram.ap(),
                            out_offset=bass.IndirectOffsetOnAxis(
                                ap=flat_i[:, :1], axis=0
                            ),
                            in_=iota_m[:, i : i + 1],
                            in_offset=None,
                            bounds_check=bc_t,
                            oob_is_err=False,
                        )
                    )
            strip_intra_deps(tscat_insts)
            t_sb = setup.tile([C, KMAX], i32)
            nc.sync.dma_start(
                t_sb[:], t_dram.ap().rearrange("(c k) o -> c (k o)", c=C)
            )

            # ---- D table as two column-half tables on different partition
            # ranges, so the scatter DMA source reads spread over all 16 SDMA
            # engines (a single 80-partition table loads only ~10 of them).
            N2 = N // 2
            tab_lo = setup.tile([C, N2], fp32)      # cols [0, N2) @ parts 0-79
            tab_hi = setup.tile([P, N2], fp32)      # cols [N2, N) @ parts 32-111
            tab_hi_st = setup.tile([C, N2], fp32)   # staging @ parts 0-79
            PSH = 32  # partition shift of the high table (PE psum base must be 0/32/64)
            for Q in range(NQ):
                pre_q = pipe.tile([P, QT, C], fp32, tag="pre")
                nc.sync.dma_start(pre_q[:], pre_tiled[:, Q * QT : (Q + 1) * QT, :])
                # softplus(x) = relu(x) + ln(1 + exp(-|x|))
                # |x| on DVE via sign-bit mask; Exp/Ln on ACT.
                t0 = pipe.tile([P, QT, C], fp32, tag="t0")
                nc.scalar.activation(t0[:], pre_q[:], AF.Abs)
                nc.scalar.activation(t0[:], t0[:], AF.Exp, scale=-1.0)
                nc.scalar.activation(t0[:], t0[:], AF.Ln, bias=1.0)
                rx = pipe.tile([P, QT, C], fp32, tag="rx")
                nc.vector.tensor_scalar_max(rx[:], pre_q[:], 0.0)
                nc.vector.tensor_add(rx[:], t0[:], rx[:])  # rx = softplus(pre)
                baseq = pipe.tile([P, QT, 1], fp32, tag="base")
                nc.vector.reduce_sum(baseq[:], rx[:], axis=mybir.AxisListType.X)
                # dtt[p, t, c] = base[p, t] - pre[p, t, c]  (onto t0)
                nc.vector.tensor_tensor(
                    out=t0[:],
                    in0=baseq[:].to_broadcast([P, QT, C]),
                    in1=pre_q[:],
                    op=ALU.subtract,
                )
                hi = Q >= NQ // 2
                for t in range(QT):
                    ps = psum.tile([P, P], fp32, tag="tr")
                    pview = ps[0:C, :]
                    nc.tensor.transpose(pview, t0[:, t, :], ident[:])
                    gt = Q * QT + t
                    if hi:
                        tp = (gt - NT // 2) * P
                        nc.scalar.copy(tab_hi_st[:, tp : tp + P], pview)
                    else:
                        tp = gt * P
                        nc.scalar.copy(tab_lo[:, tp : tp + P], pview)
                if hi:
                    # partition-shift the finished quarter to parts 32-111
                    qs = (Q - NQ // 2) * QW
                    nc.sync.dma_start(
                        tab_hi[PSH : PSH + C, qs : qs + QW],
                        tab_hi_st[:, qs : qs + QW],
                    )

            # ---- broadcast scatters: row D[c] -> out row T[c, k] ----
            mscat_insts = []

            def emit_scatter(half, k):
                src = tab_lo[:, :] if half == 0 else tab_hi[PSH : PSH + C, :]
                mscat_insts.append(
                    nc.gpsimd.indirect_dma_start(
                        out=out_dram.ap(),
                        out_offset=bass.IndirectOffsetOnAxis(
                            ap=t_sb[:, k : k + 1], axis=0
                        ),
                        in_=src,
                        in_offset=None,
                        element_offset=half * N2,
                        bounds_check=bc_m,
                        oob_is_err=False,
                    )
                )

            if not _SKIP_MAIN:
                # Front-load lo-half scatters (their table is ready first),
                # then alternate halves so both partition ranges (engine
                # sets) stream concurrently.
                LEAD = 12
                order = [(0, k) for k in range(LEAD)]
                hi_q = list(range(KMAX))
                lo_q = list(range(LEAD, KMAX))
                while hi_q or lo_q:
                    if hi_q:
                        order.append((1, hi_q.pop(0)))
                    if lo_q:
                        order.append((0, lo_q.pop(0)))
                for half, k in order:
                    emit_scatter(half, k)
            strip_intra_deps(mscat_insts)

    nc.compile()
    return nc


def _get_nc():
    global _compiled_nc
    if _compiled_nc is None:
        _compiled_nc = _build_kernel()
    return _compiled_nc


def _in_maps(gt_kind_ind, pre_cls):
    g = np.ascontiguousarray(np.asarray(gt_kind_ind).astype(np.float32))
    pre = np.ascontiguousarray(np.asarray(pre_cls, dtype=np.float32))
    assert g.shape == (M,) and pre.shape == (N, C)
    return [
        {"g": g[k * M_SHARD : (k + 1) * M_SHARD], "pre": pre}
        for k in range(N_CORES)
    ]


def kernel(gt_kind_ind, pre_cls, _trace=False):
    from concourse.bass_utils import run_bass_kernel_spmd

    nc = _get_nc()
    res = run_bass_kernel_spmd(
        nc, _in_maps(gt_kind_ind, pre_cls), list(range(N_CORES)), trace=_trace
    )
    out = np.concatenate(
        [res.results[k]["out"] for k in range(N_CORES)], axis=0
    )
    if _trace:
        return out, res
    return out


# revision 32
# speedup vs baseline: 1.5091x; 1.3089x over previous
"""Trainium2 Bass kernel for nn_CrossEntropyMoreToMore.

Math: out[i, n] = sum_c softplus(pre_cls[n, c]) - pre_cls[n, gt_kind_ind[i]]
with M = N = 8192, C = 80.

Key structure: there are only C=80 distinct output rows. Define
    D[c, n] = base[n] - pre_cls[n, c],  base[n] = sum_c softplus(pre_cls[n, c])
then out[i, :] = D[g[i], :].

Per-core plan (core k owns output rows [k*1024, (k+1)*1024)):
  1. Build D as a pair of bf16 tables (hi + lo split: D = hi + lo exactly to
     ~2^-17 relative) in [class-partition, n-free] layout, pipelined in 4
     column-quarters: load pre_cls chunk -> softplus (Abs/Exp/Ln compose) ->
     reduce -> subtract -> PE-transpose -> hi/lo split.
  2. Build a bf16 one-hot selection matrix onehotT[c, m] = (g[m] == c).
  3. For each [128 m, 512 n] psum chunk: two accumulating bf16 matmuls
     (hi then lo) produce out = onehotT.T @ D exactly in fp32 PSUM;
     2048-wide PSUM->SBUF copies alternate between DVE and ACT; 2 MB DMA
     stores stream the result to HBM.

HBM traffic per core = 32 MB output writes + 2.6 MB input reads (memory
roofline ~90 us at ~358 GB/s per core).
"""

import os

import numpy as np

M, N, C = 8192, 8192, 80
N_CORES = 8
M_SHARD = M // N_CORES  # 1024 output rows per core
P = 128  # partitions
NT = N // P  # 64 column tiles of pre_cls
MT = M_SHARD // P  # 8 m-tiles per core
NCHUNK = 512  # matmul moving-dim size (one PSUM bank of fp32)
NQ = 4  # column quarters for the pipelined table build
QT = NT // NQ  # 16 transpose tiles per quarter
QW = N // NQ  # 2048 columns per quarter

W_PSUM = 2048  # psum tile width (4 banks)
SW = 4096  # staging/store width (2 MB stores)

MM_MODE = os.environ.get("MM_MODE", "bf16")

_compiled_nc = None


def _build_kernel():
    import concourse.bacc as bacc
    import concourse.mybir as mybir
    import concourse.tile as tile
    from concourse.masks import make_identity

    nc = bacc.Bacc(
        "TRN2",
        target_bir_lowering=False,
        debug=False,
        num_devices=N_CORES,
    )
    fp32 = mybir.dt.float32
    bf16 = mybir.dt.bfloat16
    AF = mybir.ActivationFunctionType
    ALU = mybir.AluOpType

    g_dram = nc.dram_tensor("g", [M_SHARD], fp32, kind="ExternalInput")
    pre_dram = nc.dram_tensor("pre", [N, C], fp32, kind="ExternalInput")
    out_dram = nc.dram_tensor("out", [M_SHARD, N], fp32, kind="ExternalOutput")

    pre_tiled = pre_dram.ap().rearrange("(t p) c -> p t c", p=P)

    with tile.TileContext(nc) as tc:
        with (
            tc.tile_pool(name="setup", bufs=1) as setup,
            tc.tile_pool(name="pipe", bufs=2) as pipe,
            tc.tile_pool(name="stage", bufs=3) as stage,
            tc.tile_pool(name="psum", bufs=2, space="PSUM") as psum,
        ):
            ident = setup.tile([P, P], fp32)
            make_identity(nc, ident[:])

            # ---- one-hot selection matrix [80, 1024] in bf16 ----
            g_col = setup.tile([P, MT], fp32)
            nc.sync.dma_start(g_col[:], g_dram.ap().rearrange("(t p) -> p t", p=P))
            iota_row = setup.tile([P, C], fp32)
            nc.gpsimd.iota(
                iota_row[:],
                pattern=[[1, C]],
                channel_multiplier=0,
                allow_small_or_imprecise_dtypes=True,
            )
            oh = setup.tile(
                [P, M_SHARD],
                mybir.dt.float32r if MM_MODE == "f32r" else bf16,
            )
            nc.vector.memset(oh[64:P, :], 0.0)
            for i in range(MT):
                rowhot = pipe.tile([P, C], fp32, tag="rowhot")
                nc.vector.tensor_scalar(
                    out=rowhot[:],
                    in0=iota_row[:],
                    scalar1=g_col[:, i : i + 1],
                    scalar2=None,
                    op0=ALU.is_equal,
                )
                ps = psum.tile([C, P], fp32, tag="mm")
                nc.tensor.transpose(ps[:], rowhot[:], ident[:])
                nc.scalar.copy(oh[0:C, i * P : (i + 1) * P], ps[:])

            # ---- D table: bf16 hi/lo pair, or a single f32r table ----
            f32r = mybir.dt.float32r
            if MM_MODE == "f32r":
                d_hi = setup.tile([C, N], f32r)
                d_lo = None
            else:
                d_hi = setup.tile([P, N], bf16)
                d_lo = setup.tile([P, N], bf16)
                nc.vector.memset(d_hi[64:P, :], 0.0)
                nc.vector.memset(d_lo[64:P, :], 0.0)
            for Q in range(NQ):
                pre_q = pipe.tile([P, QT, C], fp32, tag="pre")
                nc.sync.dma_start(
                    pre_q[:], pre_tiled[:, Q * QT : (Q + 1) * QT, :]
                )
                # softplus(x) = relu(x) + ln(1 + exp(-|x|))
                t0 = pipe.tile([P, QT, C], fp32, tag="t0")
                nc.scalar.activation(t0[:], pre_q[:], AF.Abs)
                nc.scalar.activation(t0[:], t0[:], AF.Exp, scale=-1.0)
                nc.scalar.activation(t0[:], t0[:], AF.Ln, bias=1.0)
                rx = pipe.tile([P, QT, C], fp32, tag="rx")
                nc.vector.tensor_scalar_max(rx[:], pre_q[:], 0.0)
                nc.vector.tensor_add(rx[:], t0[:], rx[:])  # rx = softplus(pre)
                baseq = pipe.tile([P, QT, 1], fp32, tag="base")
                nc.vector.reduce_sum(baseq[:], rx[:], axis=mybir.AxisListType.X)
                # dtt[p, t, c] = base[p, t] - pre[p, t, c]  (onto t0)
                nc.vector.tensor_tensor(
                    out=t0[:],
                    in0=baseq[:].to_broadcast([P, QT, C]),
                    in1=pre_q[:],
                    op=ALU.subtract,
                )
                if MM_MODE == "f32r":
                    # ACT copies round f32 psum directly into the f32r table
                    for t in range(QT):
                        ps = psum.tile([C, P], fp32, tag="mm")
                        nc.tensor.transpose(ps[:], t0[:, t, :], ident[:])
                        tp = (Q * QT + t) * P
                        nc.scalar.copy(d_hi[:, tp : tp + P], ps[:])
                else:
                    # transpose 16 tiles into a f32 quarter, then split hi/lo
                    dt_q = pipe.tile([C, QW], fp32, tag="dtq")
                    for t in range(QT):
                        ps = psum.tile([C, P], fp32, tag="mm")
                        nc.tensor.transpose(ps[:], t0[:, t, :], ident[:])
                        nc.scalar.copy(dt_q[:, t * P : (t + 1) * P], ps[:])
                    n0 = Q * QW
                    nc.vector.tensor_copy(d_hi[0:C, n0 : n0 + QW], dt_q[:])
                    nc.vector.tensor_tensor(
                        out=d_lo[0:C, n0 : n0 + QW],
                        in0=dt_q[:],
                        in1=d_hi[0:C, n0 : n0 + QW],
                        op=ALU.subtract,
                    )

            # ---- main loop: out tile = onehot_mtile.T @ D_nchunk ----
            eng = 0
            for jo in range(N // SW):
                for i in range(MT):
                    st = stage.tile([P, SW], fp32, tag="st")
                    lhs = oh[:, i * P : (i + 1) * P]
                    for h in range(SW // W_PSUM):
                        pt = psum.tile([P, W_PSUM], fp32, tag="mm")
                        for q in range(W_PSUM // NCHUNK):
                            n0 = jo * SW + h * W_PSUM + q * NCHUNK
                            if MM_MODE == "f32r":
                                nc.tensor.matmul(
                                    pt[:, q * NCHUNK : (q + 1) * NCHUNK],
                                    lhsT=lhs,
                                    rhs=d_hi[:, n0 : n0 + NCHUNK],
                                    start=True,
                                    stop=True,
                                )
                            else:
                                nc.tensor.matmul(
                                    pt[:, q * NCHUNK : (q + 1) * NCHUNK],
                                    lhsT=lhs,
                                    rhs=d_hi[:, n0 : n0 + NCHUNK],
                                    start=True,
                                    stop=False,
                                )
                                nc.tensor.matmul(
                                    pt[:, q * NCHUNK : (q + 1) * NCHUNK],
                                    lhsT=lhs,
                                    rhs=d_lo[:, n0 : n0 + NCHUNK],
                                    start=False,
                                    stop=True,
                                )
                        dst = st[:, h * W_PSUM : (h + 1) * W_PSUM]
                        if eng % 2 == 0:
                            nc.vector.tensor_copy(dst, pt[:])
                        else:
                            nc.scalar.copy(dst, pt[:])
                        eng += 1
                    nc.sync.dma_start(
                        out_dram.ap()[i * P : (i + 1) * P, jo * SW : (jo + 1) * SW],
                        st[:],
                    )

    nc.compile()
    return nc


def _get_nc():
    global _compiled_nc
    if _compiled_nc is None:
        _compiled_nc = _build_kernel()
    return _compiled_nc


def _in_maps(gt_kind_ind, pre_cls):
    g = np.ascontiguousarray(np.asarray(gt_kind_ind).astype(np.float32))
    pre = np.ascontiguousarray(np.asarray(pre_cls, dtype=np.float32))
    assert g.shape == (M,) and pre.shape == (N, C)
    return [
        {"g": g[k * M_SHARD : (k + 1) * M_SHARD], "pre": pre}
        for k in range(N_CORES)
    ]


def kernel(gt_kind_ind, pre_cls, _trace=False):
    from concourse.bass_utils import run_bass_kernel_spmd

    nc = _get_nc()
    res = run_bass_kernel_spmd(
        nc, _in_maps(gt_kind_ind, pre_cls), list(range(N_CORES)), trace=_trace
    )
    out = np.concatenate(
        [res.results[k]["out"] for k in range(N_CORES)], axis=0
    )
    if _trace:
        return out, res
    return out


# revision 33
# speedup vs baseline: 1.5195x; 1.0069x over previous
"""Trainium2 Bass kernel for nn_CrossEntropyMoreToMore.

Math: out[i, n] = sum_c softplus(pre_cls[n, c]) - pre_cls[n, gt_kind_ind[i]]
with M = N = 8192, C = 80.

Key structure: there are only C=80 distinct output rows. Define
    D[c, n] = base[n] - pre_cls[n, c],  base[n] = sum_c softplus(pre_cls[n, c])
then out[i, :] = D[g[i], :].

Per-core plan (core k owns output rows [k*1024, (k+1)*1024)):
  1. Build D as a pair of bf16 tables (hi + lo split: D = hi + lo exactly to
     ~2^-17 relative) in [class-partition, n-free] layout, pipelined in 4
     column-quarters: load pre_cls chunk -> softplus (Abs/Exp/Ln compose) ->
     reduce -> subtract -> PE-transpose -> hi/lo split.
  2. Build a bf16 one-hot selection matrix onehotT[c, m] = (g[m] == c).
  3. For each [128 m, 512 n] psum chunk: two accumulating bf16 matmuls
     (hi then lo) produce out = onehotT.T @ D exactly in fp32 PSUM;
     2048-wide PSUM->SBUF copies alternate between DVE and ACT; 2 MB DMA
     stores stream the result to HBM.

HBM traffic per core = 32 MB output writes + 2.6 MB input reads (memory
roofline ~90 us at ~358 GB/s per core).
"""

import os

import numpy as np

M, N, C = 8192, 8192, 80
N_CORES = 8
M_SHARD = M // N_CORES  # 1024 output rows per core
P = 128  # partitions
NT = N // P  # 64 column tiles of pre_cls
MT = M_SHARD // P  # 8 m-tiles per core
NCHUNK = 512  # matmul moving-dim size (one PSUM bank of fp32)
NQ = 4  # column quarters for the pipelined table build
QT = NT // NQ  # 16 transpose tiles per quarter
QW = N // NQ  # 2048 columns per quarter

W_PSUM = 2048  # psum tile width (4 banks)
SW = 4096  # staging/store width (2 MB stores)

MM_MODE = os.environ.get("MM_MODE", "bf16")

_compiled_nc = None


def _build_kernel():
    import concourse.bacc as bacc
    import concourse.mybir as mybir
    import concourse.tile as tile
    from concourse.masks import make_identity

    nc = bacc.Bacc(
        "TRN2",
        target_bir_lowering=False,
        debug=False,
        num_devices=N_CORES,
    )
    fp32 = mybir.dt.float32
    bf16 = mybir.dt.bfloat16
    AF = mybir.ActivationFunctionType
    ALU = mybir.AluOpType

    g_dram = nc.dram_tensor("g", [M_SHARD], fp32, kind="ExternalInput")
    pre_dram = nc.dram_tensor("pre", [N, C], fp32, kind="ExternalInput")
    out_dram = nc.dram_tensor("out", [M_SHARD, N], fp32, kind="ExternalOutput")

    pre_tiled = pre_dram.ap().rearrange("(t p) c -> p t c", p=P)

    with tile.TileContext(nc) as tc:
        with (
            tc.tile_pool(name="setup", bufs=1) as setup,
            tc.tile_pool(name="pipe", bufs=2) as pipe,
            tc.tile_pool(name="stage", bufs=3) as stage,
            tc.tile_pool(name="psum", bufs=2, space="PSUM") as psum,
        ):
            ident = setup.tile([P, P], fp32)
            make_identity(nc, ident[:])

            # ---- one-hot selection matrix [80, 1024] in bf16 ----
            g_col = setup.tile([P, MT], fp32)
            nc.sync.dma_start(g_col[:], g_dram.ap().rearrange("(t p) -> p t", p=P))
            iota_row = setup.tile([P, C], fp32)
            nc.gpsimd.iota(
                iota_row[:],
                pattern=[[1, C]],
                channel_multiplier=0,
                allow_small_or_imprecise_dtypes=True,
            )
            oh = setup.tile(
                [P, M_SHARD],
                mybir.dt.float32r if MM_MODE == "f32r" else bf16,
            )
            nc.gpsimd.memset(oh[64:P, :], 0.0)
            for i in range(MT):
                rowhot = pipe.tile([P, C], fp32, tag="rowhot")
                nc.vector.tensor_scalar(
                    out=rowhot[:],
                    in0=iota_row[:],
                    scalar1=g_col[:, i : i + 1],
                    scalar2=None,
                    op0=ALU.is_equal,
                )
                ps = psum.tile([C, P], fp32, tag="mm")
                nc.tensor.transpose(ps[:], rowhot[:], ident[:])
                nc.scalar.copy(oh[0:C, i * P : (i + 1) * P], ps[:])

            # ---- D table: bf16 hi/lo pair, or a single f32r table ----
            f32r = mybir.dt.float32r
            if MM_MODE == "f32r":
                d_hi = setup.tile([C, N], f32r)
                d_lo = None
            else:
                d_hi = setup.tile([P, N], bf16)
                d_lo = setup.tile([P, N], bf16)
                nc.gpsimd.memset(d_hi[64:P, :], 0.0)
                nc.gpsimd.memset(d_lo[64:P, :], 0.0)
            for Q in range(NQ):
                pre_q = pipe.tile([P, QT, C], fp32, tag="pre")
                nc.sync.dma_start(
                    pre_q[:], pre_tiled[:, Q * QT : (Q + 1) * QT, :]
                )
                # softplus(x) = relu(x) + ln(1 + exp(-|x|))
                t0 = pipe.tile([P, QT, C], fp32, tag="t0")
                nc.scalar.activation(t0[:], pre_q[:], AF.Abs)
                nc.scalar.activation(t0[:], t0[:], AF.Exp, scale=-1.0)
                nc.scalar.activation(t0[:], t0[:], AF.Ln, bias=1.0)
                rx = pipe.tile([P, QT, C], fp32, tag="rx")
                nc.vector.tensor_scalar_max(rx[:], pre_q[:], 0.0)
                nc.vector.tensor_add(rx[:], t0[:], rx[:])  # rx = softplus(pre)
                baseq = pipe.tile([P, QT, 1], fp32, tag="base")
                nc.vector.reduce_sum(baseq[:], rx[:], axis=mybir.AxisListType.X)
                # dtt[p, t, c] = base[p, t] - pre[p, t, c]  (onto t0)
                nc.vector.tensor_tensor(
                    out=t0[:],
                    in0=baseq[:].to_broadcast([P, QT, C]),
                    in1=pre_q[:],
                    op=ALU.subtract,
                )
                if MM_MODE == "f32r":
                    # ACT copies round f32 psum directly into the f32r table
                    for t in range(QT):
                        ps = psum.tile([C, P], fp32, tag="mm")
                        nc.tensor.transpose(ps[:], t0[:, t, :], ident[:])
                        tp = (Q * QT + t) * P
                        nc.scalar.copy(d_hi[:, tp : tp + P], ps[:])
                else:
                    # transpose 16 tiles into a f32 quarter, then split hi/lo
                    dt_q = pipe.tile([C, QW], fp32, tag="dtq")
                    GB = 8  # transposes per psum group (2 banks)
                    for gb in range(QT // GB):
                        psg = psum.tile([P, GB * P], fp32, tag="mm")
                        for t0i in range(GB):
                            t = gb * GB + t0i
                            nc.tensor.transpose(
                                psg[0:C, t0i * P : (t0i + 1) * P],
                                t0[:, t, :],
                                ident[:],
                            )
                        nc.scalar.copy(
                            dt_q[:, gb * GB * P : (gb + 1) * GB * P],
                            psg[0:C, :],
                        )
                    n0 = Q * QW
                    nc.vector.tensor_copy(d_hi[0:C, n0 : n0 + QW], dt_q[:])
                    nc.vector.tensor_tensor(
                        out=d_lo[0:C, n0 : n0 + QW],
                        in0=dt_q[:],
                        in1=d_hi[0:C, n0 : n0 + QW],
                        op=ALU.subtract,
                    )

            # ---- main loop: out tile = onehot_mtile.T @ D_nchunk ----
            eng = 0
            for jo in range(N // SW):
                for i in range(MT):
                    st = stage.tile([P, SW], fp32, tag="st")
                    lhs = oh[:, i * P : (i + 1) * P]
                    for h in range(SW // W_PSUM):
                        pt = psum.tile([P, W_PSUM], fp32, tag="mm")
                        for q in range(W_PSUM // NCHUNK):
                            n0 = jo * SW + h * W_PSUM + q * NCHUNK
                            if MM_MODE == "f32r":
                                nc.tensor.matmul(
                                    pt[:, q * NCHUNK : (q + 1) * NCHUNK],
                                    lhsT=lhs,
                                    rhs=d_hi[:, n0 : n0 + NCHUNK],
                                    start=True,
                                    stop=True,
                                )
                            else:
                                nc.tensor.matmul(
                                    pt[:, q * NCHUNK : (q + 1) * NCHUNK],
                                    lhsT=lhs,
                                    rhs=d_hi[:, n0 : n0 + NCHUNK],
                                    start=True,
                                    stop=False,
                                )
                                nc.tensor.matmul(
                                    pt[:, q * NCHUNK : (q + 1) * NCHUNK],
                                    lhsT=lhs,
                                    rhs=d_lo[:, n0 : n0 + NCHUNK],
                                    start=False,
                                    stop=True,
                                )
                        dst = st[:, h * W_PSUM : (h + 1) * W_PSUM]
                        if eng % 2 == 0:
                            nc.vector.tensor_copy(dst, pt[:])
                        else:
                            nc.scalar.copy(dst, pt[:])
                        eng += 1
                    nc.sync.dma_start(
                        out_dram.ap()[i * P : (i + 1) * P, jo * SW : (jo + 1) * SW],
                        st[:],
                    )

    nc.compile()
    return nc


def _get_nc():
    global _compiled_nc
    if _compiled_nc is None:
        _compiled_nc = _build_kernel()
    return _compiled_nc


def _in_maps(gt_kind_ind, pre_cls):
    g = np.ascontiguousarray(np.asarray(gt_kind_ind).astype(np.float32))
    pre = np.ascontiguousarray(np.asarray(pre_cls, dtype=np.float32))
    assert g.shape == (M,) and pre.shape == (N, C)
    return [
        {"g": g[k * M_SHARD : (k + 1) * M_SHARD], "pre": pre}
        for k in range(N_CORES)
    ]


def kernel(gt_kind_ind, pre_cls, _trace=False):
    from concourse.bass_utils import run_bass_kernel_spmd

    nc = _get_nc()
    res = run_bass_kernel_spmd(
        nc, _in_maps(gt_kind_ind, pre_cls), list(range(N_CORES)), trace=_trace
    )
    out = np.concatenate(
        [res.results[k]["out"] for k in range(N_CORES)], axis=0
    )
    if _trace:
        return out, res
    return out


# revision 34
# speedup vs baseline: 1.5205x; 1.0006x over previous
"""Trainium2 Bass kernel for nn_CrossEntropyMoreToMore.

Math: out[i, n] = sum_c softplus(pre_cls[n, c]) - pre_cls[n, gt_kind_ind[i]]
with M = N = 8192, C = 80.

Key structure: there are only C=80 distinct output rows. Define
    D[c, n] = base[n] - pre_cls[n, c],  base[n] = sum_c softplus(pre_cls[n, c])
then out[i, :] = D[g[i], :].

Per-core plan (core k owns output rows [k*1024, (k+1)*1024)):
  1. Build D as a pair of bf16 tables (hi + lo split: D = hi + lo exactly to
     ~2^-17 relative) in [class-partition, n-free] layout, pipelined in 4
     column-quarters: load pre_cls chunk -> softplus (Abs/Exp/Ln compose) ->
     reduce -> subtract -> PE-transpose -> hi/lo split.
  2. Build a bf16 one-hot selection matrix onehotT[c, m] = (g[m] == c).
  3. For each [128 m, 512 n] psum chunk: two accumulating bf16 matmuls
     (hi then lo) produce out = onehotT.T @ D exactly in fp32 PSUM;
     2048-wide PSUM->SBUF copies alternate between DVE and ACT; 2 MB DMA
     stores stream the result to HBM.

HBM traffic per core = 32 MB output writes + 2.6 MB input reads (memory
roofline ~90 us at ~358 GB/s per core).
"""

import os

import numpy as np

M, N, C = 8192, 8192, 80
N_CORES = 8
M_SHARD = M // N_CORES  # 1024 output rows per core
P = 128  # partitions
NT = N // P  # 64 column tiles of pre_cls
MT = M_SHARD // P  # 8 m-tiles per core
NCHUNK = 512  # matmul moving-dim size (one PSUM bank of fp32)
NQ = 4  # column quarters for the pipelined table build
QT = NT // NQ  # 16 transpose tiles per quarter
QW = N // NQ  # 2048 columns per quarter

W_PSUM = 2048  # psum tile width (4 banks)
SW = 2048  # staging/store width (1 MB stores)

MM_MODE = os.environ.get("MM_MODE", "bf16")

_compiled_nc = None


def _build_kernel():
    import concourse.bacc as bacc
    import concourse.mybir as mybir
    import concourse.tile as tile
    from concourse.masks import make_identity

    nc = bacc.Bacc(
        "TRN2",
        target_bir_lowering=False,
        debug=False,
        num_devices=N_CORES,
    )
    fp32 = mybir.dt.float32
    bf16 = mybir.dt.bfloat16
    AF = mybir.ActivationFunctionType
    ALU = mybir.AluOpType

    g_dram = nc.dram_tensor("g", [M_SHARD], fp32, kind="ExternalInput")
    pre_dram = nc.dram_tensor("pre", [N, C], fp32, kind="ExternalInput")
    out_dram = nc.dram_tensor("out", [M_SHARD, N], fp32, kind="ExternalOutput")

    pre_tiled = pre_dram.ap().rearrange("(t p) c -> p t c", p=P)

    with tile.TileContext(nc) as tc:
        with (
            tc.tile_pool(name="setup", bufs=1) as setup,
            tc.tile_pool(name="pipe", bufs=2) as pipe,
            tc.tile_pool(name="stage", bufs=4) as stage,
            tc.tile_pool(name="psum", bufs=2, space="PSUM") as psum,
        ):
            ident = setup.tile([P, P], fp32)
            make_identity(nc, ident[:])

            # ---- one-hot selection matrix [80, 1024] in bf16 ----
            g_col = setup.tile([P, MT], fp32)
            nc.sync.dma_start(g_col[:], g_dram.ap().rearrange("(t p) -> p t", p=P))
            iota_row = setup.tile([P, C], fp32)
            nc.gpsimd.iota(
                iota_row[:],
                pattern=[[1, C]],
                channel_multiplier=0,
                allow_small_or_imprecise_dtypes=True,
            )
            oh = setup.tile(
                [P, M_SHARD],
                mybir.dt.float32r if MM_MODE == "f32r" else bf16,
            )
            nc.gpsimd.memset(oh[64:P, :], 0.0)
            for i in range(MT):
                rowhot = pipe.tile([P, C], fp32, tag="rowhot")
                nc.vector.tensor_scalar(
                    out=rowhot[:],
                    in0=iota_row[:],
                    scalar1=g_col[:, i : i + 1],
                    scalar2=None,
                    op0=ALU.is_equal,
                )
                ps = psum.tile([C, P], fp32, tag="mm")
                nc.tensor.transpose(ps[:], rowhot[:], ident[:])
                nc.scalar.copy(oh[0:C, i * P : (i + 1) * P], ps[:])

            # ---- D table: bf16 hi/lo pair, or a single f32r table ----
            f32r = mybir.dt.float32r
            if MM_MODE == "f32r":
                d_hi = setup.tile([C, N], f32r)
                d_lo = None
            else:
                d_hi = setup.tile([P, N], bf16)
                d_lo = setup.tile([P, N], bf16)
                nc.gpsimd.memset(d_hi[64:P, :], 0.0)
                nc.gpsimd.memset(d_lo[64:P, :], 0.0)
            for Q in range(NQ):
                pre_q = pipe.tile([P, QT, C], fp32, tag="pre")
                nc.sync.dma_start(
                    pre_q[:], pre_tiled[:, Q * QT : (Q + 1) * QT, :]
                )
                # softplus(x) = relu(x) + ln(1 + exp(-|x|))
                t0 = pipe.tile([P, QT, C], fp32, tag="t0")
                nc.scalar.activation(t0[:], pre_q[:], AF.Abs)
                nc.scalar.activation(t0[:], t0[:], AF.Exp, scale=-1.0)
                nc.scalar.activation(t0[:], t0[:], AF.Ln, bias=1.0)
                rx = pipe.tile([P, QT, C], fp32, tag="rx")
                nc.vector.tensor_scalar_max(rx[:], pre_q[:], 0.0)
                nc.vector.tensor_add(rx[:], t0[:], rx[:])  # rx = softplus(pre)
                baseq = pipe.tile([P, QT, 1], fp32, tag="base")
                nc.vector.reduce_sum(baseq[:], rx[:], axis=mybir.AxisListType.X)
                # dtt[p, t, c] = base[p, t] - pre[p, t, c]  (onto t0)
                nc.vector.tensor_tensor(
                    out=t0[:],
                    in0=baseq[:].to_broadcast([P, QT, C]),
                    in1=pre_q[:],
                    op=ALU.subtract,
                )
                if MM_MODE == "f32r":
                    # ACT copies round f32 psum directly into the f32r table
                    for t in range(QT):
                        ps = psum.tile([C, P], fp32, tag="mm")
                        nc.tensor.transpose(ps[:], t0[:, t, :], ident[:])
                        tp = (Q * QT + t) * P
                        nc.scalar.copy(d_hi[:, tp : tp + P], ps[:])
                else:
                    # transpose 16 tiles into a f32 quarter, then split hi/lo
                    dt_q = pipe.tile([C, QW], fp32, tag="dtq")
                    GB = 8  # transposes per psum group (2 banks)
                    for gb in range(QT // GB):
                        psg = psum.tile([P, GB * P], fp32, tag="mm")
                        for t0i in range(GB):
                            t = gb * GB + t0i
                            nc.tensor.transpose(
                                psg[0:C, t0i * P : (t0i + 1) * P],
                                t0[:, t, :],
                                ident[:],
                            )
                        nc.scalar.copy(
                            dt_q[:, gb * GB * P : (gb + 1) * GB * P],
                            psg[0:C, :],
                        )
                    n0 = Q * QW
                    nc.vector.tensor_copy(d_hi[0:C, n0 : n0 + QW], dt_q[:])
                    nc.vector.tensor_tensor(
                        out=d_lo[0:C, n0 : n0 + QW],
                        in0=dt_q[:],
                        in1=d_hi[0:C, n0 : n0 + QW],
                        op=ALU.subtract,
                    )

            # ---- main loop: out tile = onehot_mtile.T @ D_nchunk ----
            eng = 0
            for jo in range(N // SW):
                for i in range(MT):
                    st = stage.tile([P, SW], fp32, tag="st")
                    lhs = oh[:, i * P : (i + 1) * P]
                    for h in range(SW // W_PSUM):
                        pt = psum.tile([P, W_PSUM], fp32, tag="mm")
                        for q in range(W_PSUM // NCHUNK):
                            n0 = jo * SW + h * W_PSUM + q * NCHUNK
                            if MM_MODE == "f32r":
                                nc.tensor.matmul(
                                    pt[:, q * NCHUNK : (q + 1) * NCHUNK],
                                    lhsT=lhs,
                                    rhs=d_hi[:, n0 : n0 + NCHUNK],
                                    start=True,
                                    stop=True,
                                )
                            else:
                                nc.tensor.matmul(
                                    pt[:, q * NCHUNK : (q + 1) * NCHUNK],
                                    lhsT=lhs,
                                    rhs=d_hi[:, n0 : n0 + NCHUNK],
                                    start=True,
                                    stop=False,
                                )
                                nc.tensor.matmul(
                                    pt[:, q * NCHUNK : (q + 1) * NCHUNK],
                                    lhsT=lhs,
                                    rhs=d_lo[:, n0 : n0 + NCHUNK],
                                    start=False,
                                    stop=True,
                                )
                        dst = st[:, h * W_PSUM : (h + 1) * W_PSUM]
                        if eng % 2 == 0:
                            nc.vector.tensor_copy(dst, pt[:])
                        else:
                            nc.scalar.copy(dst, pt[:])
                        eng += 1
                    nc.sync.dma_start(
                        out_dram.ap()[i * P : (i + 1) * P, jo * SW : (jo + 1) * SW],
                        st[:],
                    )

    nc.compile()
    return nc


def _get_nc():
    global _compiled_nc
    if _compiled_nc is None:
        _compiled_nc = _build_kernel()
    return _compiled_nc


def _in_maps(gt_kind_ind, pre_cls):
    g = np.ascontiguousarray(np.asarray(gt_kind_ind).astype(np.float32))
    pre = np.ascontiguousarray(np.asarray(pre_cls, dtype=np.float32))
    assert g.shape == (M,) and pre.shape == (N, C)
    return [
        {"g": g[k * M_SHARD : (k + 1) * M_SHARD], "pre": pre}
        for k in range(N_CORES)
    ]


def kernel(gt_kind_ind, pre_cls, _trace=False):
    from concourse.bass_utils import run_bass_kernel_spmd

    nc = _get_nc()
    res = run_bass_kernel_spmd(
        nc, _in_maps(gt_kind_ind, pre_cls), list(range(N_CORES)), trace=_trace
    )
    out = np.concatenate(
        [res.results[k]["out"] for k in range(N_CORES)], axis=0
    )
    if _trace:
        return out, res
    return out


# revision 35
# speedup vs baseline: 1.6809x; 1.1055x over previous
"""Trainium2 Bass kernel for nn_CrossEntropyMoreToMore.

Math: out[i, n] = sum_c softplus(pre_cls[n, c]) - pre_cls[n, gt_kind_ind[i]]
with M = N = 8192, C = 80.

Key structure: there are only C=80 distinct output rows. Define
    D[c, n] = base[n] - pre_cls[n, c],  base[n] = sum_c softplus(pre_cls[n, c])
then out[i, :] = D[g[i], :].

Per-core plan (core k owns output rows [k*1024, (k+1)*1024)):
  1. Build D as a pair of bf16 tables (hi + lo split: D = hi + lo exactly to
     ~2^-17 relative) in [class-partition, n-free] layout, pipelined in 4
     column-quarters: load pre_cls chunk -> softplus (Abs/Exp/Ln compose) ->
     reduce -> subtract -> PE-transpose -> hi/lo split.
  2. Build a bf16 one-hot selection matrix onehotT[c, m] = (g[m] == c).
  3. For each [128 m, 512 n] psum chunk: two accumulating bf16 matmuls
     (hi then lo) produce out = onehotT.T @ D exactly in fp32 PSUM;
     2048-wide PSUM->SBUF copies alternate between DVE and ACT; 2 MB DMA
     stores stream the result to HBM.

HBM traffic per core = 32 MB output writes + 2.6 MB input reads (memory
roofline ~90 us at ~358 GB/s per core).
"""

import os

import numpy as np

M, N, C = 8192, 8192, 80
N_CORES = 8
M_SHARD = M // N_CORES  # 1024 output rows per core
P = 128  # partitions
NT = N // P  # 64 column tiles of pre_cls
MT = M_SHARD // P  # 8 m-tiles per core
NCHUNK = 512  # matmul moving-dim size (one PSUM bank of fp32)
NQ = 4  # column quarters for the pipelined table build
QT = NT // NQ  # 16 transpose tiles per quarter
QW = N // NQ  # 2048 columns per quarter

W_PSUM = 1024  # psum tile width (2 banks)
SW = 2048  # staging/store width (1 MB stores)

MM_MODE = os.environ.get("MM_MODE", "bf16")

_compiled_nc = None


def _build_kernel():
    import concourse.bacc as bacc
    import concourse.mybir as mybir
    import concourse.tile as tile
    from concourse.masks import make_identity

    nc = bacc.Bacc(
        "TRN2",
        target_bir_lowering=False,
        debug=False,
        num_devices=N_CORES,
    )
    fp32 = mybir.dt.float32
    bf16 = mybir.dt.bfloat16
    AF = mybir.ActivationFunctionType
    ALU = mybir.AluOpType

    g_dram = nc.dram_tensor("g", [M_SHARD], fp32, kind="ExternalInput")
    pre_dram = nc.dram_tensor("pre", [N, C], fp32, kind="ExternalInput")
    out_dram = nc.dram_tensor("out", [M_SHARD, N], fp32, kind="ExternalOutput")

    pre_tiled = pre_dram.ap().rearrange("(t p) c -> p t c", p=P)

    with tile.TileContext(nc) as tc:
        with (
            tc.tile_pool(name="setup", bufs=1) as setup,
            tc.tile_pool(name="pipe", bufs=2) as pipe,
            tc.tile_pool(name="stage", bufs=4) as stage,
            tc.tile_pool(name="psum", bufs=4, space="PSUM") as psum,
        ):
            ident = setup.tile([P, P], fp32)
            make_identity(nc, ident[:])

            # ---- one-hot selection matrix [80, 1024] in bf16 ----
            g_col = setup.tile([P, MT], fp32)
            nc.sync.dma_start(g_col[:], g_dram.ap().rearrange("(t p) -> p t", p=P))
            iota_row = setup.tile([P, C], fp32)
            nc.gpsimd.iota(
                iota_row[:],
                pattern=[[1, C]],
                channel_multiplier=0,
                allow_small_or_imprecise_dtypes=True,
            )
            oh = setup.tile(
                [P, M_SHARD],
                mybir.dt.float32r if MM_MODE == "f32r" else bf16,
            )
            nc.gpsimd.memset(oh[64:P, :], 0.0)
            for i in range(MT):
                rowhot = pipe.tile([P, C], fp32, tag="rowhot")
                nc.vector.tensor_scalar(
                    out=rowhot[:],
                    in0=iota_row[:],
                    scalar1=g_col[:, i : i + 1],
                    scalar2=None,
                    op0=ALU.is_equal,
                )
                ps = psum.tile([C, P], fp32, tag="mm")
                nc.tensor.transpose(ps[:], rowhot[:], ident[:])
                nc.scalar.copy(oh[0:C, i * P : (i + 1) * P], ps[:])

            # ---- D table: bf16 hi/lo pair, or a single f32r table ----
            f32r = mybir.dt.float32r
            if MM_MODE == "f32r":
                d_hi = setup.tile([C, N], f32r)
                d_lo = None
            else:
                d_hi = setup.tile([P, N], bf16)
                d_lo = setup.tile([P, N], bf16)
                nc.gpsimd.memset(d_hi[64:P, :], 0.0)
                nc.gpsimd.memset(d_lo[64:P, :], 0.0)
            for Q in range(NQ):
                pre_q = pipe.tile([P, QT, C], fp32, tag="pre")
                nc.sync.dma_start(
                    pre_q[:], pre_tiled[:, Q * QT : (Q + 1) * QT, :]
                )
                # softplus(x) = relu(x) + ln(1 + exp(-|x|))
                t0 = pipe.tile([P, QT, C], fp32, tag="t0")
                nc.scalar.activation(t0[:], pre_q[:], AF.Abs)
                nc.scalar.activation(t0[:], t0[:], AF.Exp, scale=-1.0)
                nc.scalar.activation(t0[:], t0[:], AF.Ln, bias=1.0)
                rx = pipe.tile([P, QT, C], fp32, tag="rx")
                nc.vector.tensor_scalar_max(rx[:], pre_q[:], 0.0)
                nc.vector.tensor_add(rx[:], t0[:], rx[:])  # rx = softplus(pre)
                baseq = pipe.tile([P, QT, 1], fp32, tag="base")
                nc.vector.reduce_sum(baseq[:], rx[:], axis=mybir.AxisListType.X)
                # dtt[p, t, c] = base[p, t] - pre[p, t, c]  (onto t0)
                nc.vector.tensor_tensor(
                    out=t0[:],
                    in0=baseq[:].to_broadcast([P, QT, C]),
                    in1=pre_q[:],
                    op=ALU.subtract,
                )
                if MM_MODE == "f32r":
                    # ACT copies round f32 psum directly into the f32r table
                    for t in range(QT):
                        ps = psum.tile([C, P], fp32, tag="mm")
                        nc.tensor.transpose(ps[:], t0[:, t, :], ident[:])
                        tp = (Q * QT + t) * P
                        nc.scalar.copy(d_hi[:, tp : tp + P], ps[:])
                else:
                    # transpose 16 tiles into a f32 quarter, then split hi/lo
                    dt_q = pipe.tile([C, QW], fp32, tag="dtq")
                    GB = 8  # transposes per psum group (2 banks)
                    for gb in range(QT // GB):
                        psg = psum.tile([P, GB * P], fp32, tag="mm")
                        for t0i in range(GB):
                            t = gb * GB + t0i
                            nc.tensor.transpose(
                                psg[0:C, t0i * P : (t0i + 1) * P],
                                t0[:, t, :],
                                ident[:],
                            )
                        nc.scalar.copy(
                            dt_q[:, gb * GB * P : (gb + 1) * GB * P],
                            psg[0:C, :],
                        )
                    n0 = Q * QW
                    nc.vector.tensor_copy(d_hi[0:C, n0 : n0 + QW], dt_q[:])
                    nc.vector.tensor_tensor(
                        out=d_lo[0:C, n0 : n0 + QW],
                        in0=dt_q[:],
                        in1=d_hi[0:C, n0 : n0 + QW],
                        op=ALU.subtract,
                    )

            # ---- main loop: out tile = onehot_mtile.T @ D_nchunk ----
            eng = 0
            for jo in range(N // SW):
                for i in range(MT):
                    st = stage.tile([P, SW], fp32, tag="st")
                    lhs = oh[:, i * P : (i + 1) * P]
                    for h in range(SW // W_PSUM):
                        pt = psum.tile([P, W_PSUM], fp32, tag="mm")
                        for q in range(W_PSUM // NCHUNK):
                            n0 = jo * SW + h * W_PSUM + q * NCHUNK
                            if MM_MODE == "f32r":
                                nc.tensor.matmul(
                                    pt[:, q * NCHUNK : (q + 1) * NCHUNK],
                                    lhsT=lhs,
                                    rhs=d_hi[:, n0 : n0 + NCHUNK],
                                    start=True,
                                    stop=True,
                                )
                            else:
                                nc.tensor.matmul(
                                    pt[:, q * NCHUNK : (q + 1) * NCHUNK],
                                    lhsT=lhs,
                                    rhs=d_hi[:, n0 : n0 + NCHUNK],
                                    start=True,
                                    stop=False,
                                )
                                nc.tensor.matmul(
                                    pt[:, q * NCHUNK : (q + 1) * NCHUNK],
                                    lhsT=lhs,
                                    rhs=d_lo[:, n0 : n0 + NCHUNK],
                                    start=False,
                                    stop=True,
                                )
                        dst = st[:, h * W_PSUM : (h + 1) * W_PSUM]
                        if eng % 2 == 0:
                            nc.vector.tensor_copy(dst, pt[:])
                        else:
                            nc.scalar.copy(dst, pt[:])
                        eng += 1
                    nc.sync.dma_start(
                        out_dram.ap()[i * P : (i + 1) * P, jo * SW : (jo + 1) * SW],
                        st[:],
                    )

    nc.compile()
    return nc


def _get_nc():
    global _compiled_nc
    if _compiled_nc is None:
        _compiled_nc = _build_kernel()
    return _compiled_nc


def _in_maps(gt_kind_ind, pre_cls):
    g = np.ascontiguousarray(np.asarray(gt_kind_ind).astype(np.float32))
    pre = np.ascontiguousarray(np.asarray(pre_cls, dtype=np.float32))
    assert g.shape == (M,) and pre.shape == (N, C)
    return [
        {"g": g[k * M_SHARD : (k + 1) * M_SHARD], "pre": pre}
        for k in range(N_CORES)
    ]


def kernel(gt_kind_ind, pre_cls, _trace=False):
    from concourse.bass_utils import run_bass_kernel_spmd

    nc = _get_nc()
    res = run_bass_kernel_spmd(
        nc, _in_maps(gt_kind_ind, pre_cls), list(range(N_CORES)), trace=_trace
    )
    out = np.concatenate(
        [res.results[k]["out"] for k in range(N_CORES)], axis=0
    )
    if _trace:
        return out, res
    return out


# revision 36
# speedup vs baseline: 1.6851x; 1.0025x over previous
"""Trainium2 Bass kernel for nn_CrossEntropyMoreToMore.

Math: out[i, n] = sum_c softplus(pre_cls[n, c]) - pre_cls[n, gt_kind_ind[i]]
with M = N = 8192, C = 80.

Key structure: there are only C=80 distinct output rows. Define
    D[c, n] = base[n] - pre_cls[n, c],  base[n] = sum_c softplus(pre_cls[n, c])
then out[i, :] = D[g[i], :].

Per-core plan (core k owns output rows [k*1024, (k+1)*1024)):
  1. Build D as a pair of bf16 tables (hi + lo split: D = hi + lo exactly to
     ~2^-17 relative) in [class-partition, n-free] layout, pipelined in 4
     column-quarters: load pre_cls chunk -> softplus (Abs/Exp/Ln compose) ->
     reduce -> subtract -> PE-transpose -> hi/lo split.
  2. Build a bf16 one-hot selection matrix onehotT[c, m] = (g[m] == c).
  3. For each [128 m, 512 n] psum chunk: two accumulating bf16 matmuls
     (hi then lo) produce out = onehotT.T @ D exactly in fp32 PSUM;
     2048-wide PSUM->SBUF copies alternate between DVE and ACT; 2 MB DMA
     stores stream the result to HBM.

HBM traffic per core = 32 MB output writes + 2.6 MB input reads (memory
roofline ~90 us at ~358 GB/s per core).
"""

import os

import numpy as np

M, N, C = 8192, 8192, 80
N_CORES = 8
M_SHARD = M // N_CORES  # 1024 output rows per core
P = 128  # partitions
NT = N // P  # 64 column tiles of pre_cls
MT = M_SHARD // P  # 8 m-tiles per core
NCHUNK = 512  # matmul moving-dim size (one PSUM bank of fp32)
NQ = 4  # column quarters for the pipelined table build
QT = NT // NQ  # 16 transpose tiles per quarter
QW = N // NQ  # 2048 columns per quarter

W_PSUM = 1024  # psum tile width (2 banks)
SW = 2048  # staging/store width (1 MB stores)

MM_MODE = os.environ.get("MM_MODE", "bf16")

_compiled_nc = None


def _build_kernel():
    import concourse.bacc as bacc
    import concourse.mybir as mybir
    import concourse.tile as tile
    from concourse.masks import make_identity

    nc = bacc.Bacc(
        "TRN2",
        target_bir_lowering=False,
        debug=False,
        num_devices=N_CORES,
    )
    fp32 = mybir.dt.float32
    bf16 = mybir.dt.bfloat16
    AF = mybir.ActivationFunctionType
    ALU = mybir.AluOpType

    g_dram = nc.dram_tensor("g", [M_SHARD], fp32, kind="ExternalInput")
    pre_dram = nc.dram_tensor("pre", [N, C], fp32, kind="ExternalInput")
    out_dram = nc.dram_tensor("out", [M_SHARD, N], fp32, kind="ExternalOutput")

    pre_tiled = pre_dram.ap().rearrange("(t p) c -> p t c", p=P)

    with tile.TileContext(nc) as tc:
        with (
            tc.tile_pool(name="setup", bufs=1) as setup,
            tc.tile_pool(name="pipe", bufs=2) as pipe,
            tc.tile_pool(name="stage", bufs=6) as stage,
            tc.tile_pool(name="psum", bufs=4, space="PSUM") as psum,
        ):
            ident = setup.tile([P, P], fp32)
            make_identity(nc, ident[:])

            # ---- one-hot selection matrix [80, 1024] in bf16 ----
            g_col = setup.tile([P, MT], fp32)
            nc.sync.dma_start(g_col[:], g_dram.ap().rearrange("(t p) -> p t", p=P))
            iota_row = setup.tile([P, C], fp32)
            nc.gpsimd.iota(
                iota_row[:],
                pattern=[[1, C]],
                channel_multiplier=0,
                allow_small_or_imprecise_dtypes=True,
            )
            oh = setup.tile(
                [P, M_SHARD],
                mybir.dt.float32r if MM_MODE == "f32r" else bf16,
            )
            nc.gpsimd.memset(oh[64:P, :], 0.0)
            for i in range(MT):
                rowhot = pipe.tile([P, C], fp32, tag="rowhot")
                nc.vector.tensor_scalar(
                    out=rowhot[:],
                    in0=iota_row[:],
                    scalar1=g_col[:, i : i + 1],
                    scalar2=None,
                    op0=ALU.is_equal,
                )
                ps = psum.tile([C, P], fp32, tag="mm")
                nc.tensor.transpose(ps[:], rowhot[:], ident[:])
                nc.scalar.copy(oh[0:C, i * P : (i + 1) * P], ps[:])

            # ---- D table: bf16 hi/lo pair, or a single f32r table ----
            f32r = mybir.dt.float32r
            if MM_MODE == "f32r":
                d_hi = setup.tile([C, N], f32r)
                d_lo = None
            else:
                d_hi = setup.tile([P, N], bf16)
                d_lo = setup.tile([P, N], bf16)
                nc.gpsimd.memset(d_hi[64:P, :], 0.0)
                nc.gpsimd.memset(d_lo[64:P, :], 0.0)
            for Q in range(NQ):
                pre_q = pipe.tile([P, QT, C], fp32, tag="pre")
                nc.sync.dma_start(
                    pre_q[:], pre_tiled[:, Q * QT : (Q + 1) * QT, :]
                )
                # softplus(x) = relu(x) + ln(1 + exp(-|x|))
                t0 = pipe.tile([P, QT, C], fp32, tag="t0")
                nc.scalar.activation(t0[:], pre_q[:], AF.Abs)
                nc.scalar.activation(t0[:], t0[:], AF.Exp, scale=-1.0)
                nc.scalar.activation(t0[:], t0[:], AF.Ln, bias=1.0)
                rx = pipe.tile([P, QT, C], fp32, tag="rx")
                nc.vector.tensor_scalar_max(rx[:], pre_q[:], 0.0)
                nc.vector.tensor_add(rx[:], t0[:], rx[:])  # rx = softplus(pre)
                baseq = pipe.tile([P, QT, 1], fp32, tag="base")
                nc.vector.reduce_sum(baseq[:], rx[:], axis=mybir.AxisListType.X)
                # dtt[p, t, c] = base[p, t] - pre[p, t, c]  (onto t0)
                nc.vector.tensor_tensor(
                    out=t0[:],
                    in0=baseq[:].to_broadcast([P, QT, C]),
                    in1=pre_q[:],
                    op=ALU.subtract,
                )
                if MM_MODE == "f32r":
                    # ACT copies round f32 psum directly into the f32r table
                    for t in range(QT):
                        ps = psum.tile([C, P], fp32, tag="mm")
                        nc.tensor.transpose(ps[:], t0[:, t, :], ident[:])
                        tp = (Q * QT + t) * P
                        nc.scalar.copy(d_hi[:, tp : tp + P], ps[:])
                else:
                    # transpose 16 tiles into a f32 quarter, then split hi/lo
                    dt_q = pipe.tile([C, QW], fp32, tag="dtq")
                    GB = 8  # transposes per psum group (2 banks)
                    for gb in range(QT // GB):
                        psg = psum.tile([P, GB * P], fp32, tag="mm")
                        for t0i in range(GB):
                            t = gb * GB + t0i
                            nc.tensor.transpose(
                                psg[0:C, t0i * P : (t0i + 1) * P],
                                t0[:, t, :],
                                ident[:],
                            )
                        nc.scalar.copy(
                            dt_q[:, gb * GB * P : (gb + 1) * GB * P],
                            psg[0:C, :],
                        )
                    n0 = Q * QW
                    nc.vector.tensor_copy(d_hi[0:C, n0 : n0 + QW], dt_q[:])
                    nc.vector.tensor_tensor(
                        out=d_lo[0:C, n0 : n0 + QW],
                        in0=dt_q[:],
                        in1=d_hi[0:C, n0 : n0 + QW],
                        op=ALU.subtract,
                    )

            # ---- main loop: out tile = onehot_mtile.T @ D_nchunk ----
            eng = 0
            for jo in range(N // SW):
                for i in range(MT):
                    st = stage.tile([P, SW], fp32, tag="st")
                    lhs = oh[:, i * P : (i + 1) * P]
                    for h in range(SW // W_PSUM):
                        pt = psum.tile([P, W_PSUM], fp32, tag="mm")
                        for q in range(W_PSUM // NCHUNK):
                            n0 = jo * SW + h * W_PSUM + q * NCHUNK
                            if MM_MODE == "f32r":
                                nc.tensor.matmul(
                                    pt[:, q * NCHUNK : (q + 1) * NCHUNK],
                                    lhsT=lhs,
                                    rhs=d_hi[:, n0 : n0 + NCHUNK],
                                    start=True,
                                    stop=True,
                                )
                            else:
                                nc.tensor.matmul(
                                    pt[:, q * NCHUNK : (q + 1) * NCHUNK],
                                    lhsT=lhs,
                                    rhs=d_hi[:, n0 : n0 + NCHUNK],
                                    start=True,
                                    stop=False,
                                )
                                nc.tensor.matmul(
                                    pt[:, q * NCHUNK : (q + 1) * NCHUNK],
                                    lhsT=lhs,
                                    rhs=d_lo[:, n0 : n0 + NCHUNK],
                                    start=False,
                                    stop=True,
                                )
                        dst = st[:, h * W_PSUM : (h + 1) * W_PSUM]
                        if eng % 2 == 0:
                            nc.vector.tensor_copy(dst, pt[:])
                        else:
                            nc.scalar.copy(dst, pt[:])
                        eng += 1
                    st_eng = nc.sync if (jo * MT + i) % 2 == 0 else nc.scalar
                    st_eng.dma_start(
                        out_dram.ap()[i * P : (i + 1) * P, jo * SW : (jo + 1) * SW],
                        st[:],
                    )

    nc.compile()
    return nc


def _get_nc():
    global _compiled_nc
    if _compiled_nc is None:
        _compiled_nc = _build_kernel()
    return _compiled_nc


def _in_maps(gt_kind_ind, pre_cls):
    g = np.ascontiguousarray(np.asarray(gt_kind_ind).astype(np.float32))
    pre = np.ascontiguousarray(np.asarray(pre_cls, dtype=np.float32))
    assert g.shape == (M,) and pre.shape == (N, C)
    return [
        {"g": g[k * M_SHARD : (k + 1) * M_SHARD], "pre": pre}
        for k in range(N_CORES)
    ]


def kernel(gt_kind_ind, pre_cls, _trace=False):
    from concourse.bass_utils import run_bass_kernel_spmd

    nc = _get_nc()
    res = run_bass_kernel_spmd(
        nc, _in_maps(gt_kind_ind, pre_cls), list(range(N_CORES)), trace=_trace
    )
    out = np.concatenate(
        [res.results[k]["out"] for k in range(N_CORES)], axis=0
    )
    if _trace:
        return out, res
    return out
